# revision 61
# baseline (speedup 1.0000x reference)
"""Two-layer GCN (ClinicalGCN) on 8 Trainium2 NeuronCores.

Math (fold the symmetric GCN norm into node features; self-loops handled
algebraically, not gathered; b1/b2 handled separately, and when they are
zero — as in this problem — fused away):
    h_hat[v]   = (x @ W1)[v] * dinv[v]
    agg1'[i]   = sum_{real e: dst=i} h_hat[src[e]]      (segment sum)
    h1_hat[v]  = dinv[v] * relu(dinv[v]*(agg1'[v] + h_hat[v]) + b1)
    agg2'[i]   = sum_{real e: dst=i} h1_hat[src[e]]
    out[i]     = (dinv[i]*(agg2'[i] + h1_hat[i])) @ W2 + b2

Device mapping:
  - dst-shard nodes across 8 cores; per-core 49 blocks of 128 dst nodes.
  - Feature tables are AllGather'd in TWO halves (split by source partition
    p<64 / p>=64) so the second collective overlaps the first half's
    gathers; each gather phase runs as two passes (A-half then B-half) with
    an SBUF partial-aggregate buffer carrying pass-A sums.
  - Source rows are fetched with gpsimd.dma_gather (int16 indices into the
    25088-row half-tables).  Gather calls round-robin across 4 SWDGE queues
    so descriptor generation runs on all 8 Q7 cores (queue q uses core pair
    2q/2q+1) instead of serializing on cores 0-1.
  - Trailing padding indices are -1: the gather ucode truncates trailing
    negatives, so padding costs no descriptor-generation or DMA time.
    Padding slots have sel==0 so stale msg data is multiplied by zero
    (msg buffers are memset once at startup so stale data is never NaN).
  - Table rows are partition-major within a core (row = p*nblk + j for
    local node j*128+p) making the SBUF->DRAM table writes fully
    contiguous per partition; similarly the final 'out' is written
    partition-major in one DMA and un-permuted on the host.
  - Per 128-edge chunk, a 0/1 selection matrix S (built with one DVE
    is_equal per block-half) routes messages to dst rows via PE matmul
    accumulation in PSUM.
"""

import math

import ml_dtypes
import numpy as np

import concourse.bacc as bacc
import concourse.bass as bass
import concourse.mybir as mybir
import concourse.tile as tile
from concourse.bass_utils import run_bass_kernel_spmd

P = 128
N_CORES = 8
N_QUEUES = 4
BF16 = ml_dtypes.bfloat16


class Cfg:
    def __init__(self, n_nodes, n_in, n_hid, n_out, n_cores=N_CORES):
        assert n_nodes % n_cores == 0
        self.n = n_nodes
        self.nin = n_in
        self.nh = n_hid
        self.nc_out = n_out
        self.cores = n_cores
        self.shard = n_nodes // n_cores           # real nodes per core
        self.nblk = (self.shard + P - 1) // P     # dst blocks per core
        self.pshard = self.nblk * P               # padded nodes per core
        self.j0 = (self.nblk + 1) // 2            # blocks in table half A
        self.hshardA = self.j0 * P                # rows per core, half A
        self.hshardB = (self.nblk - self.j0) * P
        self.htabA = self.hshardA * n_cores       # rows per half-table
        self.htabB = self.hshardB * n_cores
        assert max(self.htabA, self.htabB) <= 32768, "int16 gather idx limit"
        self.kin = n_in // P                      # k chunks for x @ W1


FULL = Cfg(50000, 256, 128, 4)


# ---------------------------------------------------------------- host prep
def host_prep(cfg: Cfg, x, edge_index, W1, b1, W2, b2):
    """Build per-core input arrays. Pure numpy."""
    n = cfg.n
    # degrees/norm include the self loop (GCN: deg = indeg + 1)
    dst_all = np.concatenate([edge_index[1], np.arange(n, dtype=np.int64)])
    deg = np.bincount(dst_all, minlength=n).astype(np.float32)
    dinv = np.where(deg > 0, 1.0 / np.sqrt(deg), 0.0).astype(np.float32)

    # only real edges are gathered; self loops are added algebraically
    src = edge_index[0].astype(np.int64)
    dst = edge_index[1].astype(np.int64)

    # source placement: core, partition p, block j; table halves split by
    # BLOCK range (j<j0 -> half A) so each half can be written and
    # AllGather'd as soon as its blocks are computed.  Within a half, rows
    # are partition-major (row = core*hshard + p*njh + j') so table writes
    # are contiguous per partition.
    score = src // cfg.shard
    sloc = src % cfg.shard
    sp = sloc % P
    sj = sloc // P
    half_e = (sj >= cfg.j0).astype(np.int64)
    njA, njB = cfg.j0, cfg.nblk - cfg.j0
    hrow = np.where(
        half_e == 0,
        score * cfg.hshardA + sp * njA + sj,
        score * cfg.hshardB + sp * njB + (sj - cfg.j0))     # [E]

    # order edges by destination; dst = core*shard + local so this groups
    # by (core, block) with our local block definition
    order = np.argsort(dst, kind="stable")
    dst_s = dst[order]
    hrow_s = hrow[order]
    half_s = half_e[order]
    ldl_s = dst_s % cfg.shard
    lslot_s = (ldl_s % P).astype(np.float32)
    blk_s = (dst_s // cfg.shard) * cfg.nblk + ldl_s // P

    nblk_total = cfg.cores * cfg.nblk
    # chunk counts per (block, half); K per LOCAL block = max across cores
    # (the SPMD program is shared, so per-block sizes must agree per core)
    cnt = np.zeros((nblk_total, 2), dtype=np.int64)
    np.add.at(cnt, (blk_s, half_s), 1)
    cnt3 = cnt.reshape(cfg.cores, cfg.nblk, 2)
    # shared valid-index count per (block, half): max across cores (the
    # SPMD program passes this as num_idxs_reg, so it must agree per core)
    Vmax = np.maximum(1, cnt3.max(axis=0))            # [nblk, 2]
    KH = [np.maximum(1, np.ceil(Vmax[:, h] / P)).astype(int)
          for h in range(2)]  # each: [nblk]

    # bucket sort edges by (block, half)
    key = blk_s * 2 + half_s
    order2 = np.argsort(key, kind="stable")
    hrow2 = hrow_s[order2]
    lslot2 = lslot_s[order2]
    key2 = key[order2]
    starts = np.searchsorted(key2, np.arange(nblk_total * 2 + 1))

    # column-major packed layouts: one resident SBUF tile per array, sliced
    # per block on device (avoids thousands of small per-block DMA loads)
    Ksum = KH[0] + KH[1]
    gcol = [np.concatenate([[0], np.cumsum(KH[h] * 8)]) for h in range(2)]
    lcol = np.concatenate([[0], np.cumsum(Ksum)])

    per_core = []
    for c in range(cfg.cores):
        gidx = [np.zeros((P, gcol[h][-1]), dtype=np.int16) for h in range(2)]
        ldst = np.zeros((P, lcol[-1]), dtype=BF16)
        for b in range(cfg.nblk):
            g = c * cfg.nblk + b
            ld_b = np.full((P, Ksum[b]), -1.0, dtype=np.float32)
            for h in range(2):
                lo, hi = starts[g * 2 + h], starts[g * 2 + h + 1]
                cnt_e = hi - lo
                tr = hrow2[lo:hi]
                ls = lslot2[lo:hi]
                # [real | dummy 0s up to shared Vmax | -1 padding]: the
                # gather ucode truncates trailing negatives, so slots past
                # Vmax cost no descgen/DMA time; dummy 0s keep the valid
                # count identical across cores (num_idxs_reg is shared)
                idx = np.full(KH[h][b] * P, -1, dtype=np.int16)
                idx[:cnt_e] = tr
                idx[cnt_e:Vmax[b, h]] = 0
                wrapped = idx.reshape(KH[h][b] * 8, 16).T   # [16, K*8]
                gidx[h][:, gcol[h][b]:gcol[h][b + 1]] = \
                    np.tile(wrapped, (8, 1))                # replicate
                t = np.arange(cnt_e)
                j0 = 0 if h == 0 else KH[0][b]
                ld_b[t % P, j0 + t // P] = ls
            ldst[:, lcol[b]:lcol[b + 1]] = ld_b.astype(BF16)
        xs = x[c * cfg.shard:(c + 1) * cfg.shard]
        xT = np.zeros((cfg.nin, cfg.pshard), dtype=BF16)
        xT[:, :cfg.shard] = xs.T.astype(BF16)
        dvflat = np.zeros(cfg.pshard, dtype=np.float32)
        dvflat[:cfg.shard] = dinv[c * cfg.shard:(c + 1) * cfg.shard]
        dv = dvflat.reshape(cfg.nblk, P).T.copy()   # [P, nblk]
        per_core.append({
            "xT": xT,
            "dinv": dv,
            "dinv2": dv * dv,
            "gidxA": gidx[0],
            "gidxB": gidx[1],
            "ldst": ldst,
        })

    KmaxH = int(max(KH[0].max(), KH[1].max()))
    iota = np.broadcast_to(np.arange(P, dtype=np.float32).astype(BF16),
                           (P, P))
    iota_big = np.tile(iota, (1, KmaxH)).copy()   # [P, KmaxH*P]
    ident = np.eye(P, dtype=np.float32).astype(BF16)
    shared = {
        "W1": W1.astype(BF16),
        "W2": W2.astype(BF16),
        "b1r": np.broadcast_to(b1.astype(np.float32), (P, cfg.nh)).copy(),
        "b2r": np.broadcast_to(b2.astype(np.float32), (P, cfg.nc_out)).copy(),
        "iotab": iota_big,
        "ident": ident,
    }
    in_maps = [{**shared, **pc} for pc in per_core]
    zero_bias = not (np.any(b1) or np.any(b2))
    return in_maps, (KH[0], KH[1], Vmax), zero_bias


# --------------------------------------------------------------- bass build
def build_nc(cfg: Cfg, KH, zero_bias):
    f32 = mybir.dt.float32
    bf16 = mybir.dt.bfloat16
    i16 = mybir.dt.int16
    KA, KB, Vmax = KH                # per-block chunk counts, [nblk] each
    Ksum = [int(KA[b] + KB[b]) for b in range(cfg.nblk)]
    gcolA = np.concatenate([[0], np.cumsum(np.asarray(KA) * 8)])
    gcolB = np.concatenate([[0], np.cumsum(np.asarray(KB) * 8)])
    lcol = np.concatenate([[0], np.cumsum(np.asarray(Ksum))])
    KmaxH = int(max(max(KA), max(KB)))

    nc = bacc.Bacc("TRN2", target_bir_lowering=False, debug=False,
                   num_devices=cfg.cores, num_swdge_queues=N_QUEUES)

    xT = nc.dram_tensor("xT", [cfg.nin, cfg.pshard], bf16,
                        kind="ExternalInput")
    W1 = nc.dram_tensor("W1", [cfg.nin, cfg.nh], bf16, kind="ExternalInput")
    W2 = nc.dram_tensor("W2", [cfg.nh, cfg.nc_out], bf16, kind="ExternalInput")
    b1r = nc.dram_tensor("b1r", [P, cfg.nh], f32, kind="ExternalInput")
    b2r = nc.dram_tensor("b2r", [P, cfg.nc_out], f32, kind="ExternalInput")
    dinv = nc.dram_tensor("dinv", [P, cfg.nblk], f32, kind="ExternalInput")
    dinv2 = nc.dram_tensor("dinv2", [P, cfg.nblk], f32, kind="ExternalInput")
    iotab = nc.dram_tensor("iotab", [P, KmaxH * P], bf16,
                           kind="ExternalInput")
    ident = nc.dram_tensor("ident", [P, P], bf16, kind="ExternalInput")
    gidxA = nc.dram_tensor("gidxA", [P, int(gcolA[-1])], i16,
                           kind="ExternalInput")
    gidxB = nc.dram_tensor("gidxB", [P, int(gcolB[-1])], i16,
                           kind="ExternalInput")
    ldst = nc.dram_tensor("ldst", [P, int(lcol[-1])], bf16,
                          kind="ExternalInput")
    out = nc.dram_tensor("out", [cfg.pshard, cfg.nc_out], f32,
                         kind="ExternalOutput")

    qctr = [0]

    def next_q():
        q = qctr[0] % N_QUEUES
        qctr[0] += 1
        return q

    with tile.TileContext(nc) as tc:
        with (
            tc.tile_pool(name="const", bufs=1) as cpool,
            tc.tile_pool(name="h", bufs=3) as hpool,
            tc.tile_pool(name="sel", bufs=4) as spool,
            tc.tile_pool(name="ps", bufs=4, space="PSUM") as pspool,
            tc.tile_pool(name="ps2", bufs=2, space="PSUM") as ps2pool,
            tc.tile_pool(name="dram", bufs=1, space="DRAM") as dram,
        ):
            # ---- constants in SBUF (W1 as kin slices of [128, nh])
            w1t = cpool.tile([P, cfg.kin * cfg.nh], bf16, tag="w1")
            nc.sync.dma_start(
                out=w1t[:].rearrange("p (a d) -> p a d", a=cfg.kin),
                in_=W1[:].rearrange("(a p) d -> p a d", p=P))
            # whole xT resident in SBUF: [128, kin, pshard] bf16
            xall = cpool.tile([P, cfg.kin * cfg.pshard], bf16, tag="xall")
            nc.sync.dma_start(
                out=xall[:].rearrange("p (a d) -> p a d", a=cfg.kin),
                in_=xT[:].rearrange("(a p) d -> p a d", p=P))
            w2t = cpool.tile([cfg.nh, cfg.nc_out], bf16, tag="w2")
            nc.sync.dma_start(out=w2t[:], in_=W2[:])
            b1t = cpool.tile([P, cfg.nh], f32, tag="b1")
            nc.sync.dma_start(out=b1t[:], in_=b1r[:])
            b2t = cpool.tile([P, cfg.nc_out], f32, tag="b2")
            nc.sync.dma_start(out=b2t[:], in_=b2r[:])
            iot = cpool.tile([P, KmaxH * P], bf16, tag="iotab")
            nc.sync.dma_start(out=iot[:], in_=iotab[:])
            idt = cpool.tile([P, P], bf16, tag="ident")
            nc.sync.dma_start(out=idt[:], in_=ident[:])
            dvt = cpool.tile([P, cfg.nblk], f32, tag="dinv")
            nc.sync.dma_start(out=dvt[:], in_=dinv[:])
            dv2t = cpool.tile([P, cfg.nblk], f32, tag="dinv2")
            nc.sync.dma_start(out=dv2t[:], in_=dinv2[:])

            # resident h_hat / h1_hat blocks (self-loop terms), partial
            # aggregates from pass A, and the output accumulator
            hhall = cpool.tile([P, cfg.nblk * cfg.nh], bf16, tag="hhall")
            h1all = cpool.tile([P, cfg.nblk * cfg.nh], bf16, tag="h1all")
            aggbuf = cpool.tile([P, cfg.nblk * cfg.nh], f32, tag="aggbuf")
            outall = cpool.tile([P, cfg.nblk * cfg.nc_out], f32, tag="outall")

            # resident gather indices and dst-slot arrays (used both layers)
            giA = cpool.tile([P, int(gcolA[-1])], i16, tag="giA")
            nc.sync.dma_start(out=giA[:], in_=gidxA[:])
            giB = cpool.tile([P, int(gcolB[-1])], i16, tag="giB")
            nc.sync.dma_start(out=giB[:], in_=gidxB[:])
            ldall = cpool.tile([P, int(lcol[-1])], bf16, tag="ldall")
            nc.sync.dma_start(out=ldall[:], in_=ldst[:])

            hshA = dram.tile([cfg.hshardA, cfg.nh], bf16)
            hshB = dram.tile([cfg.hshardB, cfg.nh], bf16)
            htabA = dram.tile([cfg.htabA, cfg.nh], bf16, addr_space="Shared")
            htabB = dram.tile([cfg.htabB, cfg.nh], bf16, addr_space="Shared")
            h1shA = dram.tile([cfg.hshardA, cfg.nh], bf16)
            h1shB = dram.tile([cfg.hshardB, cfg.nh], bf16)
            h1tabA = dram.tile([cfg.htabA, cfg.nh], bf16, addr_space="Shared")
            h1tabB = dram.tile([cfg.htabB, cfg.nh], bf16, addr_space="Shared")

            # Persistent msg buffers, zero-filled once: with -1 index padding
            # the gather skips padding slots, so stale buffer contents must be
            # finite (sel==0 kills them in the matmul, but 0*NaN would be NaN).
            NMSG = 6
            msgbufs = []
            for i in range(NMSG):
                mz = cpool.tile([P, KmaxH * cfg.nh], bf16, tag=f"msgb{i}")
                nc.vector.memset(mz[:], 0.0)
                msgbufs.append(mz)
            mctr = [0]

            def half_write(dst_dram, src_tile, c0, nj):
                # contiguous partition-major half-table write (block columns
                # [c0, c0+nj) of a [P, nblk*nh] tile)
                nc.sync.dma_start(
                    out=dst_dram[:].rearrange("(p j) f -> p (j f)", p=P),
                    in_=src_tile[:, c0 * cfg.nh:(c0 + nj) * cfg.nh])

            def allgather(src, dstt):
                nc.gpsimd.collective_compute(
                    "AllGather", mybir.AluOpType.bypass,
                    replica_groups=[list(range(cfg.cores))],
                    ins=[src.opt()], outs=[dstt.opt()])

            # ---------------- phase 1: h_hat = (x @ W1) * dinv -> AllGather
            # half-A table ships as soon as blocks [0, j0) are done, so the
            # first collective overlaps the rest of phase 1
            for t in range(cfg.nblk):
                ps = pspool.tile([P, cfg.nh], f32, tag="ps_agg")
                for kk in range(cfg.kin):
                    nc.tensor.matmul(
                        out=ps[:],
                        lhsT=xall[:, kk * cfg.pshard + t * P:
                                  kk * cfg.pshard + (t + 1) * P],
                        rhs=w1t[:, kk * cfg.nh:(kk + 1) * cfg.nh],
                        start=(kk == 0), stop=(kk == cfg.kin - 1))
                nc.scalar.activation(
                    out=hhall[:, t * cfg.nh:(t + 1) * cfg.nh], in_=ps[:],
                    func=mybir.ActivationFunctionType.Copy,
                    scale=dvt[:, t:t + 1])
                if t == cfg.j0 - 1:
                    half_write(hshA, hhall, 0, cfg.j0)
                    allgather(hshA, htabA)
            half_write(hshB, hhall, cfg.j0, cfg.nblk - cfg.j0)
            allgather(hshB, htabB)

            # gather + segment-sum for one (block, half) -> psum [P, nh] f32
            def gather_half(b, h, table):
                if h == 0:
                    KHh, gi, gc, j0 = int(KA[b]), giA, gcolA, 0
                else:
                    KHh, gi, gc, j0 = int(KB[b]), giB, gcolB, int(KA[b])
                msg = msgbufs[mctr[0] % NMSG]
                mctr[0] += 1
                nc.gpsimd.dma_gather(
                    out_ap=msg[:, :KHh * cfg.nh]
                    .rearrange("p (k f) -> p k f", k=KHh),
                    in_ap=table[:],
                    idxs_ap=gi[:, int(gc[b]):int(gc[b + 1])],
                    num_idxs=KHh * P,
                    num_idxs_reg=int(Vmax[b, h]),
                    elem_size=cfg.nh,
                    single_packet=False,
                    queue_num=next_q())
                sel = spool.tile([P, KmaxH * P], bf16, tag="sel")
                nc.vector.tensor_tensor(
                    out=sel[:, :KHh * P].rearrange("p (k f) -> p k f", k=KHh),
                    in0=ldall[:, int(lcol[b]) + j0:int(lcol[b]) + j0 + KHh,
                              None].to_broadcast([P, KHh, P]),
                    in1=iot[:, :KHh * P].rearrange("p (k f) -> p k f", k=KHh),
                    op=mybir.AluOpType.is_equal)
                ps = pspool.tile([P, cfg.nh], f32, tag="ps_agg")
                for j in range(KHh):
                    nc.tensor.matmul(
                        out=ps[:], lhsT=sel[:, j * P:(j + 1) * P],
                        rhs=msg[:, j * cfg.nh:(j + 1) * cfg.nh],
                        start=(j == 0), stop=(j == KHh - 1))
                return ps

            def gather_phase(tabA, tabB, selfall, epilogue, after_b=None):
                # pass A: aggbuf[b] = aggA + self-loop term
                for b in range(cfg.nblk):
                    ps = gather_half(b, 0, tabA)
                    nc.vector.tensor_add(
                        out=aggbuf[:, b * cfg.nh:(b + 1) * cfg.nh],
                        in0=ps[:],
                        in1=selfall[:, b * cfg.nh:(b + 1) * cfg.nh])
                # pass B: full agg = aggB + aggbuf[b] -> epilogue
                for b in range(cfg.nblk):
                    ps = gather_half(b, 1, tabB)
                    t0 = hpool.tile([P, cfg.nh], f32, tag="t0")
                    nc.vector.tensor_add(
                        out=t0[:], in0=ps[:],
                        in1=aggbuf[:, b * cfg.nh:(b + 1) * cfg.nh])
                    epilogue(b, t0)
                    if after_b is not None:
                        after_b(b)

            # ---------------- phase 2: h1_hat table
            def epi2(b, t0):
                h1slice = h1all[:, b * cfg.nh:(b + 1) * cfg.nh]
                if zero_bias:
                    # h1_hat = dinv^2 * relu(agg) = relu(dinv^2 * agg)
                    nc.scalar.activation(
                        out=h1slice, in_=t0[:],
                        func=mybir.ActivationFunctionType.Relu,
                        scale=dv2t[:, b:b + 1])
                else:
                    nc.vector.tensor_scalar_mul(out=t0[:], in0=t0[:],
                                                scalar1=dvt[:, b:b + 1])
                    nc.vector.tensor_add(out=t0[:], in0=t0[:], in1=b1t[:])
                    nc.vector.tensor_scalar(
                        out=h1slice, in0=t0[:], scalar1=0.0,
                        scalar2=dvt[:, b:b + 1],
                        op0=mybir.AluOpType.max, op1=mybir.AluOpType.mult)

            def after2(b):
                # ship half-A of the h1 table as soon as blocks [0, j0) have
                # their epilogues; the collective overlaps the rest of pass B
                if b == cfg.j0 - 1:
                    half_write(h1shA, h1all, 0, cfg.j0)
                    allgather(h1shA, h1tabA)

            gather_phase(htabA, htabB, hhall, epi2, after_b=after2)
            half_write(h1shB, h1all, cfg.j0, cfg.nblk - cfg.j0)
            allgather(h1shB, h1tabB)

            # ---------------- phase 3: out = (dinv*(agg2'+h1_hat)) @ W2 (+b2)
            def epi3(b, t0):
                c1 = hpool.tile([P, cfg.nh], bf16, tag="c1")
                nc.scalar.activation(out=c1[:], in_=t0[:],
                                     func=mybir.ActivationFunctionType.Copy,
                                     scale=dvt[:, b:b + 1])
                pst = ps2pool.tile([P, cfg.nh], bf16, tag="ps_t")
                nc.tensor.transpose(out=pst[:], in_=c1[:], identity=idt[:])
                aggT = hpool.tile([P, cfg.nh], bf16, tag="aggT")
                nc.scalar.copy(out=aggT[:], in_=pst[:])
                pso = ps2pool.tile([P, cfg.nc_out], f32, tag="ps_o")
                nc.tensor.matmul(out=pso[:], lhsT=aggT[:], rhs=w2t[:],
                                 start=True, stop=True)
                oslice = outall[:, b * cfg.nc_out:(b + 1) * cfg.nc_out]
                if zero_bias:
                    nc.scalar.copy(out=oslice, in_=pso[:])
                else:
                    nc.vector.tensor_add(out=oslice, in0=pso[:], in1=b2t[:])

            gather_phase(h1tabA, h1tabB, h1all, epi3)

            # single bulk output write; rows are partition-major (p*nblk+j),
            # the host driver un-permutes
            nc.sync.dma_start(
                out=out[:].rearrange("(p j) c -> p (j c)", p=P),
                in_=outall[:])

    nc.compile()
    return nc


# ------------------------------------------------------------------ driver
def unpermute_out(arr, cfg):
    """Device 'out' rows are partition-major (p*nblk+j); restore node order."""
    arr = np.asarray(arr).reshape(P, cfg.nblk, cfg.nc_out).transpose(1, 0, 2)
    return arr.reshape(cfg.pshard, cfg.nc_out)[:cfg.shard]


def kernel(x, edge_index, W1, b1, W2, b2):
    cfg = FULL
    assert x.shape == (cfg.n, cfg.nin)
    in_maps, KH, zero_bias = host_prep(
        cfg, np.asarray(x), np.asarray(edge_index), np.asarray(W1),
        np.asarray(b1), np.asarray(W2), np.asarray(b2))
    nc = build_nc(cfg, KH, zero_bias)
    res = run_bass_kernel_spmd(nc, in_maps, core_ids=list(range(cfg.cores)))
    parts = [unpermute_out(res.results[c]["out"], cfg)
             for c in range(cfg.cores)]
    return np.concatenate(parts, axis=0).astype(np.float32)


# revision 63
# speedup vs baseline: 1.0787x; 1.0787x over previous
"""Two-layer GCN (ClinicalGCN) on 8 Trainium2 NeuronCores.

Math (fold the symmetric GCN norm into node features; self-loops handled
algebraically, not gathered; b1/b2 handled separately, and when they are
zero — as in this problem — fused away):
    h_hat[v]   = (x @ W1)[v] * dinv[v]
    agg1'[i]   = sum_{real e: dst=i} h_hat[src[e]]      (segment sum)
    h1_hat[v]  = dinv[v] * relu(dinv[v]*(agg1'[v] + h_hat[v]) + b1)
    agg2'[i]   = sum_{real e: dst=i} h1_hat[src[e]]
    out[i]     = (dinv[i]*(agg2'[i] + h1_hat[i])) @ W2 + b2

Device mapping:
  - dst-shard nodes across 8 cores; per-core 49 blocks of 128 dst nodes.
  - Feature tables are AllGather'd in TWO halves (split by source partition
    p<64 / p>=64) so the second collective overlaps the first half's
    gathers; each gather phase runs as two passes (A-half then B-half) with
    an SBUF partial-aggregate buffer carrying pass-A sums.
  - Source rows are fetched with gpsimd.dma_gather (int16 indices into the
    25088-row half-tables).  Gather calls round-robin across 4 SWDGE queues
    so descriptor generation runs on all 8 Q7 cores (queue q uses core pair
    2q/2q+1) instead of serializing on cores 0-1.
  - Trailing padding indices are -1: the gather ucode truncates trailing
    negatives, so padding costs no descriptor-generation or DMA time.
    Padding slots have sel==0 so stale msg data is multiplied by zero
    (msg buffers are memset once at startup so stale data is never NaN).
  - Table rows are partition-major within a core (row = p*nblk + j for
    local node j*128+p) making the SBUF->DRAM table writes fully
    contiguous per partition; similarly the final 'out' is written
    partition-major in one DMA and un-permuted on the host.
  - Per 128-edge chunk, a 0/1 selection matrix S (built with one DVE
    is_equal per block-half) routes messages to dst rows via PE matmul
    accumulation in PSUM.
"""

import math

import ml_dtypes
import numpy as np

import concourse.bacc as bacc
import concourse.bass as bass
import concourse.mybir as mybir
import concourse.tile as tile
from concourse.bass_utils import run_bass_kernel_spmd

P = 128
N_CORES = 8
N_QUEUES = 4
BF16 = ml_dtypes.bfloat16


class Cfg:
    def __init__(self, n_nodes, n_in, n_hid, n_out, n_cores=N_CORES):
        assert n_nodes % n_cores == 0
        self.n = n_nodes
        self.nin = n_in
        self.nh = n_hid
        self.nc_out = n_out
        self.cores = n_cores
        self.shard = n_nodes // n_cores           # real nodes per core
        self.nblk = (self.shard + P - 1) // P     # dst blocks per core
        self.pshard = self.nblk * P               # padded nodes per core
        self.j0 = (self.nblk + 1) // 2            # blocks in table half A
        self.hshardA = self.j0 * P                # rows per core, half A
        self.hshardB = (self.nblk - self.j0) * P
        self.htabA = self.hshardA * n_cores       # rows per half-table
        self.htabB = self.hshardB * n_cores
        assert max(self.htabA, self.htabB) <= 32768, "int16 gather idx limit"
        self.kin = n_in // P                      # k chunks for x @ W1


FULL = Cfg(50000, 256, 128, 4)


# ---------------------------------------------------------------- host prep
def host_prep(cfg: Cfg, x, edge_index, W1, b1, W2, b2):
    """Build per-core input arrays. Pure numpy."""
    n = cfg.n
    # degrees/norm include the self loop (GCN: deg = indeg + 1)
    dst_all = np.concatenate([edge_index[1], np.arange(n, dtype=np.int64)])
    deg = np.bincount(dst_all, minlength=n).astype(np.float32)
    dinv = np.where(deg > 0, 1.0 / np.sqrt(deg), 0.0).astype(np.float32)

    # only real edges are gathered; self loops are added algebraically
    src = edge_index[0].astype(np.int64)
    dst = edge_index[1].astype(np.int64)

    # source placement: core, partition p, block j; table halves split by
    # BLOCK range (j<j0 -> half A) so each half can be written and
    # AllGather'd as soon as its blocks are computed.  Within a half, rows
    # are partition-major (row = core*hshard + p*njh + j') so table writes
    # are contiguous per partition.
    score = src // cfg.shard
    sloc = src % cfg.shard
    sp = sloc % P
    sj = sloc // P
    half_e = (sj >= cfg.j0).astype(np.int64)
    njA, njB = cfg.j0, cfg.nblk - cfg.j0
    hrow = np.where(
        half_e == 0,
        score * cfg.hshardA + sp * njA + sj,
        score * cfg.hshardB + sp * njB + (sj - cfg.j0))     # [E]

    # order edges by destination; dst = core*shard + local so this groups
    # by (core, block) with our local block definition
    order = np.argsort(dst, kind="stable")
    dst_s = dst[order]
    hrow_s = hrow[order]
    half_s = half_e[order]
    ldl_s = dst_s % cfg.shard
    lslot_s = (ldl_s % P).astype(np.float32)
    blk_s = (dst_s // cfg.shard) * cfg.nblk + ldl_s // P

    nblk_total = cfg.cores * cfg.nblk
    # chunk counts per (block, half); K per LOCAL block = max across cores
    # (the SPMD program is shared, so per-block sizes must agree per core)
    cnt = np.zeros((nblk_total, 2), dtype=np.int64)
    np.add.at(cnt, (blk_s, half_s), 1)
    cnt3 = cnt.reshape(cfg.cores, cfg.nblk, 2)
    # shared valid-index count per (block, half): max across cores (the
    # SPMD program passes this as num_idxs_reg, so it must agree per core)
    Vmax = np.maximum(1, cnt3.max(axis=0))            # [nblk, 2]
    KH = [np.maximum(1, np.ceil(Vmax[:, h] / P)).astype(int)
          for h in range(2)]  # each: [nblk]

    # bucket sort edges by (block, half)
    key = blk_s * 2 + half_s
    order2 = np.argsort(key, kind="stable")
    hrow2 = hrow_s[order2]
    lslot2 = lslot_s[order2]
    key2 = key[order2]
    starts = np.searchsorted(key2, np.arange(nblk_total * 2 + 1))

    # column-major packed layouts: one resident SBUF tile per array, sliced
    # per block on device (avoids thousands of small per-block DMA loads)
    Ksum = KH[0] + KH[1]
    gcol = [np.concatenate([[0], np.cumsum(KH[h] * 8)]) for h in range(2)]
    lcol = np.concatenate([[0], np.cumsum(Ksum)])

    per_core = []
    for c in range(cfg.cores):
        gidx = [np.zeros((P, gcol[h][-1]), dtype=np.int16) for h in range(2)]
        ldst = np.zeros((P, lcol[-1]), dtype=BF16)
        for b in range(cfg.nblk):
            g = c * cfg.nblk + b
            ld_b = np.full((P, Ksum[b]), -1.0, dtype=np.float32)
            for h in range(2):
                lo, hi = starts[g * 2 + h], starts[g * 2 + h + 1]
                cnt_e = hi - lo
                tr = hrow2[lo:hi]
                ls = lslot2[lo:hi]
                # [real | dummy 0s up to shared Vmax | -1 padding]: the
                # gather ucode truncates trailing negatives, so slots past
                # Vmax cost no descgen/DMA time; dummy 0s keep the valid
                # count identical across cores (num_idxs_reg is shared)
                idx = np.full(KH[h][b] * P, -1, dtype=np.int16)
                idx[:cnt_e] = tr
                idx[cnt_e:Vmax[b, h]] = 0
                wrapped = idx.reshape(KH[h][b] * 8, 16).T   # [16, K*8]
                gidx[h][:, gcol[h][b]:gcol[h][b + 1]] = \
                    np.tile(wrapped, (8, 1))                # replicate
                t = np.arange(cnt_e)
                j0 = 0 if h == 0 else KH[0][b]
                ld_b[t % P, j0 + t // P] = ls
            ldst[:, lcol[b]:lcol[b + 1]] = ld_b.astype(BF16)
        xs = x[c * cfg.shard:(c + 1) * cfg.shard]
        xT = np.zeros((cfg.nin, cfg.pshard), dtype=BF16)
        xT[:, :cfg.shard] = xs.T.astype(BF16)
        dvflat = np.zeros(cfg.pshard, dtype=np.float32)
        dvflat[:cfg.shard] = dinv[c * cfg.shard:(c + 1) * cfg.shard]
        dv = dvflat.reshape(cfg.nblk, P).T.copy()   # [P, nblk]
        per_core.append({
            "xT": xT,
            "dinv": dv,
            "dinv2": dv * dv,
            "gidxA": gidx[0],
            "gidxB": gidx[1],
            "ldst": ldst,
        })

    KmaxH = int(max(KH[0].max(), KH[1].max()))
    iota = np.broadcast_to(np.arange(P, dtype=np.float32).astype(BF16),
                           (P, P))
    iota_big = np.tile(iota, (1, KmaxH)).copy()   # [P, KmaxH*P]
    ident = np.eye(P, dtype=np.float32).astype(BF16)
    shared = {
        "W1": W1.astype(BF16),
        "W2": W2.astype(BF16),
        "b1r": np.broadcast_to(b1.astype(np.float32), (P, cfg.nh)).copy(),
        "b2r": np.broadcast_to(b2.astype(np.float32), (P, cfg.nc_out)).copy(),
        "iotab": iota_big,
        "ident": ident,
    }
    in_maps = [{**shared, **pc} for pc in per_core]
    zero_bias = not (np.any(b1) or np.any(b2))
    return in_maps, (KH[0], KH[1], Vmax), zero_bias


# --------------------------------------------------------------- bass build
def build_nc(cfg: Cfg, KH, zero_bias):
    f32 = mybir.dt.float32
    bf16 = mybir.dt.bfloat16
    i16 = mybir.dt.int16
    KA, KB, Vmax = KH                # per-block chunk counts, [nblk] each
    Ksum = [int(KA[b] + KB[b]) for b in range(cfg.nblk)]
    gcolA = np.concatenate([[0], np.cumsum(np.asarray(KA) * 8)])
    gcolB = np.concatenate([[0], np.cumsum(np.asarray(KB) * 8)])
    lcol = np.concatenate([[0], np.cumsum(np.asarray(Ksum))])
    KmaxH = int(max(max(KA), max(KB)))

    nc = bacc.Bacc("TRN2", target_bir_lowering=False, debug=False,
                   num_devices=cfg.cores, num_swdge_queues=N_QUEUES)

    xT = nc.dram_tensor("xT", [cfg.nin, cfg.pshard], bf16,
                        kind="ExternalInput")
    W1 = nc.dram_tensor("W1", [cfg.nin, cfg.nh], bf16, kind="ExternalInput")
    W2 = nc.dram_tensor("W2", [cfg.nh, cfg.nc_out], bf16, kind="ExternalInput")
    b1r = nc.dram_tensor("b1r", [P, cfg.nh], f32, kind="ExternalInput")
    b2r = nc.dram_tensor("b2r", [P, cfg.nc_out], f32, kind="ExternalInput")
    dinv = nc.dram_tensor("dinv", [P, cfg.nblk], f32, kind="ExternalInput")
    dinv2 = nc.dram_tensor("dinv2", [P, cfg.nblk], f32, kind="ExternalInput")
    iotab = nc.dram_tensor("iotab", [P, KmaxH * P], bf16,
                           kind="ExternalInput")
    ident = nc.dram_tensor("ident", [P, P], bf16, kind="ExternalInput")
    gidxA = nc.dram_tensor("gidxA", [P, int(gcolA[-1])], i16,
                           kind="ExternalInput")
    gidxB = nc.dram_tensor("gidxB", [P, int(gcolB[-1])], i16,
                           kind="ExternalInput")
    ldst = nc.dram_tensor("ldst", [P, int(lcol[-1])], bf16,
                          kind="ExternalInput")
    out = nc.dram_tensor("out", [cfg.pshard, cfg.nc_out], f32,
                         kind="ExternalOutput")

    qctr = [0]

    def next_q():
        q = qctr[0] % N_QUEUES
        qctr[0] += 1
        return q

    with tile.TileContext(nc) as tc:
        with (
            tc.tile_pool(name="const", bufs=1) as cpool,
            tc.tile_pool(name="h", bufs=3) as hpool,
            tc.tile_pool(name="sel", bufs=4) as spool,
            tc.tile_pool(name="ps", bufs=4, space="PSUM") as pspool,
            tc.tile_pool(name="ps2", bufs=2, space="PSUM") as ps2pool,
            tc.tile_pool(name="dram", bufs=1, space="DRAM") as dram,
        ):
            # ---- constants in SBUF (W1 as kin slices of [128, nh])
            w1t = cpool.tile([P, cfg.kin * cfg.nh], bf16, tag="w1")
            nc.sync.dma_start(
                out=w1t[:].rearrange("p (a d) -> p a d", a=cfg.kin),
                in_=W1[:].rearrange("(a p) d -> p a d", p=P))
            # whole xT resident in SBUF: [128, kin, pshard] bf16
            xall = cpool.tile([P, cfg.kin * cfg.pshard], bf16, tag="xall")
            nc.sync.dma_start(
                out=xall[:].rearrange("p (a d) -> p a d", a=cfg.kin),
                in_=xT[:].rearrange("(a p) d -> p a d", p=P))
            w2t = cpool.tile([cfg.nh, cfg.nc_out], bf16, tag="w2")
            nc.sync.dma_start(out=w2t[:], in_=W2[:])
            b1t = cpool.tile([P, cfg.nh], f32, tag="b1")
            nc.sync.dma_start(out=b1t[:], in_=b1r[:])
            b2t = cpool.tile([P, cfg.nc_out], f32, tag="b2")
            nc.sync.dma_start(out=b2t[:], in_=b2r[:])
            iot = cpool.tile([P, KmaxH * P], bf16, tag="iotab")
            nc.sync.dma_start(out=iot[:], in_=iotab[:])
            idt = cpool.tile([P, P], bf16, tag="ident")
            nc.sync.dma_start(out=idt[:], in_=ident[:])
            dvt = cpool.tile([P, cfg.nblk], f32, tag="dinv")
            nc.sync.dma_start(out=dvt[:], in_=dinv[:])
            dv2t = cpool.tile([P, cfg.nblk], f32, tag="dinv2")
            nc.sync.dma_start(out=dv2t[:], in_=dinv2[:])

            # resident h_hat / h1_hat blocks (self-loop terms), partial
            # aggregates from pass A, and the output accumulator
            hhall = cpool.tile([P, cfg.nblk * cfg.nh], bf16, tag="hhall")
            h1all = cpool.tile([P, cfg.nblk * cfg.nh], bf16, tag="h1all")
            aggbuf = cpool.tile([P, cfg.nblk * cfg.nh], f32, tag="aggbuf")
            outall = cpool.tile([P, cfg.nblk * cfg.nc_out], f32, tag="outall")

            # resident gather indices and dst-slot arrays (used both layers)
            giA = cpool.tile([P, int(gcolA[-1])], i16, tag="giA")
            nc.sync.dma_start(out=giA[:], in_=gidxA[:])
            giB = cpool.tile([P, int(gcolB[-1])], i16, tag="giB")
            nc.sync.dma_start(out=giB[:], in_=gidxB[:])
            ldall = cpool.tile([P, int(lcol[-1])], bf16, tag="ldall")
            nc.sync.dma_start(out=ldall[:], in_=ldst[:])

            hshA = dram.tile([cfg.hshardA, cfg.nh], bf16)
            hshB = dram.tile([cfg.hshardB, cfg.nh], bf16)
            htabA = dram.tile([cfg.htabA, cfg.nh], bf16, addr_space="Shared")
            htabB = dram.tile([cfg.htabB, cfg.nh], bf16, addr_space="Shared")
            h1shA = dram.tile([cfg.hshardA, cfg.nh], bf16)
            h1shB = dram.tile([cfg.hshardB, cfg.nh], bf16)
            h1tabA = dram.tile([cfg.htabA, cfg.nh], bf16, addr_space="Shared")
            h1tabB = dram.tile([cfg.htabB, cfg.nh], bf16, addr_space="Shared")

            # warmup: a dummy gather loads the Q7 'mlp' library and warms the
            # SWDGE path during the head instead of after the first AllGather
            wti = cpool.tile([P, 8], i16, tag="warmi")
            nc.vector.memset(wti[:], 0)
            wto = cpool.tile([P, cfg.nh], bf16, tag="warmo")
            nc.gpsimd.dma_gather(
                out_ap=wto[:].rearrange("p (k f) -> p k f", k=1),
                in_ap=ident[:],
                idxs_ap=wti[:],
                num_idxs=P, num_idxs_reg=P,
                elem_size=cfg.nh, single_packet=False, queue_num=next_q())

            # Persistent msg buffers, zero-filled once: with -1 index padding
            # the gather skips padding slots, so stale buffer contents must be
            # finite (sel==0 kills them in the matmul, but 0*NaN would be NaN).
            NMSG = 6
            msgbufs = []
            for i in range(NMSG):
                mz = cpool.tile([P, KmaxH * cfg.nh], bf16, tag=f"msgb{i}")
                nc.vector.memset(mz[:], 0.0)
                msgbufs.append(mz)
            mctr = [0]

            def half_write(dst_dram, src_tile, c0, nj):
                # contiguous partition-major half-table write (block columns
                # [c0, c0+nj) of a [P, nblk*nh] tile)
                nc.sync.dma_start(
                    out=dst_dram[:].rearrange("(p j) f -> p (j f)", p=P),
                    in_=src_tile[:, c0 * cfg.nh:(c0 + nj) * cfg.nh])

            def allgather(src, dstt):
                nc.gpsimd.collective_compute(
                    "AllGather", mybir.AluOpType.bypass,
                    replica_groups=[list(range(cfg.cores))],
                    ins=[src.opt()], outs=[dstt.opt()])

            # ---------------- phase 1: h_hat = (x @ W1) * dinv -> AllGather
            # half-A table ships as soon as blocks [0, j0) are done, so the
            # first collective overlaps the rest of phase 1
            for t in range(cfg.nblk):
                ps = pspool.tile([P, cfg.nh], f32, tag="ps_agg")
                for kk in range(cfg.kin):
                    nc.tensor.matmul(
                        out=ps[:],
                        lhsT=xall[:, kk * cfg.pshard + t * P:
                                  kk * cfg.pshard + (t + 1) * P],
                        rhs=w1t[:, kk * cfg.nh:(kk + 1) * cfg.nh],
                        start=(kk == 0), stop=(kk == cfg.kin - 1))
                nc.scalar.activation(
                    out=hhall[:, t * cfg.nh:(t + 1) * cfg.nh], in_=ps[:],
                    func=mybir.ActivationFunctionType.Copy,
                    scale=dvt[:, t:t + 1])
                if t == cfg.j0 - 1:
                    half_write(hshA, hhall, 0, cfg.j0)
                    allgather(hshA, htabA)
            half_write(hshB, hhall, cfg.j0, cfg.nblk - cfg.j0)
            allgather(hshB, htabB)

            # gather + segment-sum for one (block, half) -> psum [P, nh] f32
            def gather_half(b, h, table):
                if h == 0:
                    KHh, gi, gc, j0 = int(KA[b]), giA, gcolA, 0
                else:
                    KHh, gi, gc, j0 = int(KB[b]), giB, gcolB, int(KA[b])
                msg = msgbufs[mctr[0] % NMSG]
                mctr[0] += 1
                nc.gpsimd.dma_gather(
                    out_ap=msg[:, :KHh * cfg.nh]
                    .rearrange("p (k f) -> p k f", k=KHh),
                    in_ap=table[:],
                    idxs_ap=gi[:, int(gc[b]):int(gc[b + 1])],
                    num_idxs=KHh * P,
                    num_idxs_reg=int(Vmax[b, h]),
                    elem_size=cfg.nh,
                    single_packet=False,
                    queue_num=next_q())
                sel = spool.tile([P, KmaxH * P], bf16, tag="sel")
                nc.vector.tensor_tensor(
                    out=sel[:, :KHh * P].rearrange("p (k f) -> p k f", k=KHh),
                    in0=ldall[:, int(lcol[b]) + j0:int(lcol[b]) + j0 + KHh,
                              None].to_broadcast([P, KHh, P]),
                    in1=iot[:, :KHh * P].rearrange("p (k f) -> p k f", k=KHh),
                    op=mybir.AluOpType.is_equal)
                ps = pspool.tile([P, cfg.nh], f32, tag="ps_agg")
                for j in range(KHh):
                    nc.tensor.matmul(
                        out=ps[:], lhsT=sel[:, j * P:(j + 1) * P],
                        rhs=msg[:, j * cfg.nh:(j + 1) * cfg.nh],
                        start=(j == 0), stop=(j == KHh - 1))
                return ps

            def gather_phase(tabA, tabB, selfall, epilogue, after_b=None):
                # pass A: aggbuf[b] = aggA + self-loop term
                for b in range(cfg.nblk):
                    ps = gather_half(b, 0, tabA)
                    nc.vector.tensor_add(
                        out=aggbuf[:, b * cfg.nh:(b + 1) * cfg.nh],
                        in0=ps[:],
                        in1=selfall[:, b * cfg.nh:(b + 1) * cfg.nh])
                # pass B: full agg = aggB + aggbuf[b] -> epilogue
                for b in range(cfg.nblk):
                    ps = gather_half(b, 1, tabB)
                    t0 = hpool.tile([P, cfg.nh], f32, tag="t0")
                    nc.vector.tensor_add(
                        out=t0[:], in0=ps[:],
                        in1=aggbuf[:, b * cfg.nh:(b + 1) * cfg.nh])
                    epilogue(b, t0)
                    if after_b is not None:
                        after_b(b)

            # ---------------- phase 2: h1_hat table
            def epi2(b, t0):
                h1slice = h1all[:, b * cfg.nh:(b + 1) * cfg.nh]
                if zero_bias:
                    # h1_hat = dinv^2 * relu(agg) = relu(dinv^2 * agg)
                    nc.scalar.activation(
                        out=h1slice, in_=t0[:],
                        func=mybir.ActivationFunctionType.Relu,
                        scale=dv2t[:, b:b + 1])
                else:
                    nc.vector.tensor_scalar_mul(out=t0[:], in0=t0[:],
                                                scalar1=dvt[:, b:b + 1])
                    nc.vector.tensor_add(out=t0[:], in0=t0[:], in1=b1t[:])
                    nc.vector.tensor_scalar(
                        out=h1slice, in0=t0[:], scalar1=0.0,
                        scalar2=dvt[:, b:b + 1],
                        op0=mybir.AluOpType.max, op1=mybir.AluOpType.mult)

            def after2(b):
                # ship half-A of the h1 table as soon as blocks [0, j0) have
                # their epilogues; the collective overlaps the rest of pass B
                if b == cfg.j0 - 1:
                    half_write(h1shA, h1all, 0, cfg.j0)
                    allgather(h1shA, h1tabA)

            gather_phase(htabA, htabB, hhall, epi2, after_b=after2)
            half_write(h1shB, h1all, cfg.j0, cfg.nblk - cfg.j0)
            allgather(h1shB, h1tabB)

            # ---------------- phase 3: out = (dinv*(agg2'+h1_hat)) @ W2 (+b2)
            def epi3(b, t0):
                c1 = hpool.tile([P, cfg.nh], bf16, tag="c1")
                nc.scalar.activation(out=c1[:], in_=t0[:],
                                     func=mybir.ActivationFunctionType.Copy,
                                     scale=dvt[:, b:b + 1])
                pst = ps2pool.tile([P, cfg.nh], bf16, tag="ps_t")
                nc.tensor.transpose(out=pst[:], in_=c1[:], identity=idt[:])
                aggT = hpool.tile([P, cfg.nh], bf16, tag="aggT")
                nc.scalar.copy(out=aggT[:], in_=pst[:])
                pso = ps2pool.tile([P, cfg.nc_out], f32, tag="ps_o")
                nc.tensor.matmul(out=pso[:], lhsT=aggT[:], rhs=w2t[:],
                                 start=True, stop=True)
                oslice = outall[:, b * cfg.nc_out:(b + 1) * cfg.nc_out]
                if zero_bias:
                    nc.scalar.copy(out=oslice, in_=pso[:])
                else:
                    nc.vector.tensor_add(out=oslice, in0=pso[:], in1=b2t[:])

            gather_phase(h1tabA, h1tabB, h1all, epi3)

            # single bulk output write; rows are partition-major (p*nblk+j),
            # the host driver un-permutes
            nc.sync.dma_start(
                out=out[:].rearrange("(p j) c -> p (j c)", p=P),
                in_=outall[:])

    nc.compile()
    return nc


# ------------------------------------------------------------------ driver
def unpermute_out(arr, cfg):
    """Device 'out' rows are partition-major (p*nblk+j); restore node order."""
    arr = np.asarray(arr).reshape(P, cfg.nblk, cfg.nc_out).transpose(1, 0, 2)
    return arr.reshape(cfg.pshard, cfg.nc_out)[:cfg.shard]


def kernel(x, edge_index, W1, b1, W2, b2):
    cfg = FULL
    assert x.shape == (cfg.n, cfg.nin)
    in_maps, KH, zero_bias = host_prep(
        cfg, np.asarray(x), np.asarray(edge_index), np.asarray(W1),
        np.asarray(b1), np.asarray(W2), np.asarray(b2))
    nc = build_nc(cfg, KH, zero_bias)
    res = run_bass_kernel_spmd(nc, in_maps, core_ids=list(range(cfg.cores)))
    parts = [unpermute_out(res.results[c]["out"], cfg)
             for c in range(cfg.cores)]
    return np.concatenate(parts, axis=0).astype(np.float32)
